# revision 93
# baseline (speedup 1.0000x reference)
"""Causal self-attention (dense transformer) on 8 TRN2 NeuronCores.

Sharding: heads+batch tensor-parallel. Each core c owns 2 heads (2c, 2c+1)
for all 4 batches:
  - QKV projection with the w_qkv row-slice for its heads (x is replicated,
    fed pre-transposed as xT [d, tokens] so d lands on SBUF partitions).
  - Causal attention for its 8 (batch, head) pairs in "transposed scores"
    layout S_t[tk, tq]. PV is computed flipped (stationary = P subtile,
    moving = V with an appended ones column) so y comes out token-major with
    the softmax denominator as an extra output column; normalization is then
    a cheap per-partition broadcast multiply, and a PE transpose returns y to
    feature-major for the exchange.
  - Per-batch AllToAll exchanges y feature-slices for token-slices (issued as
    each batch's attention finishes so collectives overlap later batches'
    compute), then each core runs the output projection for its token chunks.
Host side: shard/transpose/cast inputs, concat+transpose the output.

Compute dtype bf16 (PSUM accumulation fp32), storage fp32 in/out.
"""

import numpy as np
import ml_dtypes

import concourse.bass as bass
import concourse.mybir as mybir
import concourse.tile as tile
from concourse.bass_utils import run_bass_kernel_spmd

BF16 = mybir.dt.bfloat16
F32 = mybir.dt.float32
AF = mybir.ActivationFunctionType

# Full-size problem constants (hardcoded per harness contract)
N_CORES = 8
BSZ, SEQ, D, N_HEAD = 4, 2048, 1024, 16
HD = 64  # head dim


def _split_multi_waits(nc):
    """walrus on this build accepts at most ONE sync-wait command per
    instruction. Hoist extra waits onto standalone same-engine nops placed
    immediately before the instruction (queue order preserves semantics)."""
    edits = []
    for func in nc.m.functions:
        for bb in func.blocks:
            insts = bb.instructions
            for idx, ins in enumerate(insts):
                si = ins.sync_info
                if si is not None and len(si.on_wait) > 1:
                    edits.append((bb, idx, ins))
    for bb, idx, ins in reversed(edits):
        si = ins.sync_info
        extra, keep = list(si.on_wait[:-1]), [si.on_wait[-1]]
        ins.sync_info = mybir.SyncInfo(on_wait=keep, on_update=list(si.on_update))
        nops = []
        for w in extra:
            nop = nc.engines[ins.engine].nop().ins
            host = nc.cur_bb.bb.instructions
            assert host[-1] is nop
            host.pop()
            nop.sync_info = mybir.SyncInfo(on_wait=[w], on_update=[])
            nops.append(nop)
        live = bb.instructions
        for j, nop in enumerate(nops):
            live.insert(idx + j, nop)


def build_nc(n_cores=N_CORES, bsz=BSZ, seq=SEQ, d=D, n_head=N_HEAD):
    hd = HD
    hpc = n_head // n_cores          # heads per core
    fl = hpc * hd                    # local feature width (q/k/v per core)
    T = bsz * seq                    # total tokens
    kd = d // 128                    # contraction tiles over d
    tb = min(512, seq)               # tq block width (matmul free dim)
    nqb = seq // tb                  # q-blocks per batch
    dtiles = tb // 128               # 128-tiles per q-block (diag masks)
    nt = T // 128                    # total 128-token tiles
    scale = float(1.0 / np.sqrt(hd))

    tsb = seq // n_cores             # per-batch token chunk per core
    nc = bass.Bass(num_devices=n_cores)
    xT = nc.declare_dram_parameter("xT", [d, T], BF16, isOutput=False)
    wqkv = nc.declare_dram_parameter("wqkv", [d, 3 * fl], BF16, isOutput=False)
    wproj = nc.declare_dram_parameter("wproj", [d, d], BF16, isOutput=False)
    out = nc.declare_dram_parameter("out", [d, bsz * tsb], F32, isOutput=True)
    # every batch exchanges token-major: y ships straight from the normalize
    # buffer; feature-major transposes run after the exchange as tail work.
    a2a_in = [nc.dram_tensor(f"a2a_in{b}", [n_cores, tsb, fl], BF16)
              for b in range(bsz)]
    a2a_out = [nc.dram_tensor(f"a2a_out{b}", [n_cores, tsb, fl], BF16)
               for b in range(bsz)]

    with tile.TileContext(nc) as tc:
        with (
            tc.tile_pool(name="const", bufs=1) as const,
            tc.tile_pool(name="xin", bufs=2) as xin,
            tc.tile_pool(name="work", bufs=3) as work,
            tc.tile_pool(name="psum", bufs=1, space="PSUM") as psum,
        ):
            # ---- persistent SBUF ----
            w_sb = [const.tile([128, 3 * fl], BF16, name=f"w_sb{i}") for i in range(kd)]
            for i in range(kd):
                nc.scalar.dma_start(w_sb[i][:], wqkv[i * 128:(i + 1) * 128, :])
            wp_sb = [const.tile([fl, d], BF16, name=f"wp_sb{i}") for i in range(n_cores)]

            q_sb = const.tile([fl, T], BF16, name="q_sb")
            k_sb = const.tile([fl, T], BF16, name="k_sb")
            y_sb = const.tile([fl, T], BF16, name="y_sb")
            # v token-major with a ones column per (tile, head): tile g, head h
            # occupies cols g*hpc*(hd+1) + h*(hd+1) + [0, hd+1); col hd is ones.
            vw = hpc * (hd + 1)
            v_sb = const.tile([128, nt * vw], BF16, name="v_sb")
            ones_ap = v_sb.rearrange("p (n h c) -> p n h c", h=hpc,
                                     c=hd + 1)[:, :, :, hd:hd + 1]
            nc.vector.memset(ones_ap, 1.0)

            # triangular mask [128,128]: keep S_t[tk_i, tq_j] iff i <= j
            tri = const.tile([128, 128], BF16, name="tri")
            nc.gpsimd.memset(tri[:], 1.0)
            nc.gpsimd.affine_select(
                out=tri[:], in_=tri[:],
                compare_op=mybir.AluOpType.is_ge, fill=0.0,
                base=0, channel_multiplier=-1, pattern=[[1, 128]],
            )
            # identity [128,128] for PE transposes
            ident = const.tile([128, 128], BF16, name="ident")
            nc.gpsimd.memset(ident[:], 1.0)
            nc.gpsimd.affine_select(
                out=ident[:], in_=ident[:],
                compare_op=mybir.AluOpType.is_equal, fill=0.0,
                base=0, channel_multiplier=-1, pattern=[[1, 128]],
            )

            # ---- unified (batch, q-block) stream with qkv prefetch ----
            # pending: FIFO of small PE work closures (qkv projection and
            # output projection chunks) drained between attention k-tiles so
            # the in-order PE queue always has work while exp (ACT) paces the
            # attention chain.
            pending = []       # qkv chunks: must finish before their unit
            pending_proj = []  # proj chunks: tail filler, drained lazily

            def drain(n=None):
                k = len(pending) if n is None else min(n, len(pending))
                for _ in range(k):
                    pending.pop(0)()

            def issue_x_dma(b, qb, split_x=False):
                tbi = b * (seq // tb) + qb
                ts0 = tbi * tb
                x_t = xin.tile([128, kd * tb], BF16, name="x_t", tag="x", bufs=4)
                xsrc = xT.rearrange("(i p) T -> p i T", p=128)[:, :, ts0:ts0 + tb]
                if split_x:
                    # first block: four DMAs so early kd tiles land sooner
                    q = kd // 4
                    for j in range(4):
                        nc.sync.dma_start(
                            x_t[:].rearrange("p (i t) -> p i t",
                                             i=kd)[:, j * q:(j + 1) * q],
                            xsrc[:, j * q:(j + 1) * q])
                else:
                    # one DMA: in [kd, 128, tb] (d-tiles) -> out [128, kd, tb]
                    nc.sync.dma_start(
                        x_t[:].rearrange("p (i t) -> p i t", i=kd), xsrc)
                return x_t

            def push_qkv_chunks(b, qb, x_t):
                tbi = b * (seq // tb) + qb
                ts0 = tbi * tb
                # q, k (feature-major): out [fl, tb]; two chunks each
                for which, dst in ((0, q_sb), (1, k_sb)):
                    st = {}
                    qc = kd // 4

                    def qk_part(j, which=which, dst=dst, st=st, qc=qc):
                        if j == 0:
                            st["ps"] = psum.tile([fl, tb], F32,
                                                 name=f"ps_qk{which}",
                                                 tag="mm512", bufs=2)
                        ps = st["ps"]
                        for i in range(j * qc, (j + 1) * qc):
                            nc.tensor.matmul(
                                ps[:], w_sb[i][:, which * fl:(which + 1) * fl],
                                x_t[:, i * tb:(i + 1) * tb],
                                start=(i == 0), stop=(i == kd - 1))
                        if j == 3:
                            nc.vector.tensor_copy(dst[:, ts0:ts0 + tb], ps[:])

                    for j in range(4):
                        pending.append(lambda j=j, f=qk_part: f(j))
                # v (token-major): out [128 tok, fl]; two chunks per tt
                for tt in range(dtiles):
                    vst = {}

                    def v_part(j, tt=tt, vst=vst):
                        gti = tbi * dtiles + tt
                        if j == 0:
                            vst["ps"] = psum.tile([128, fl], F32, name="ps_v",
                                                  tag="mm512", bufs=2)
                        ps_v = vst["ps"]
                        for i in range(j * (kd // 2), (j + 1) * (kd // 2)):
                            nc.tensor.matmul(
                                ps_v[:],
                                x_t[:, i * tb + tt * 128:i * tb + (tt + 1) * 128],
                                w_sb[i][:, 2 * fl:3 * fl],
                                start=(i == 0), stop=(i == kd - 1))
                        if j == 1:
                            nc.vector.tensor_copy(
                                v_sb.rearrange("p (n h c) -> p n h c", h=hpc,
                                               c=hd + 1)[:, gti, :, 0:hd],
                                vst["ps"][:].rearrange("p (h c) -> p h c", c=hd))

                    for j in range(2):
                        pending.append(lambda j=j, f=v_part: f(j))

            def qk_scores(b, qb, tki):
                # scores for k-tile tki of unit (b, qb), both heads
                tq0 = b * seq + qb * tb
                t0 = b * seq + tki * 128
                m = tki - qb * dtiles
                c0 = 128 * m if m > 0 else 0
                ps_s = psum.tile([128, hpc * tb], F32, name="ps_s",
                                 tag="s2", bufs=2)
                p_t = work.tile([128, hpc * tb], BF16, name="p_t",
                                tag="pt", bufs=6)
                for h in range(hpc):
                    hs = slice(h * hd, (h + 1) * hd)
                    nc.tensor.matmul(ps_s[:, h * tb + c0:(h + 1) * tb],
                                     k_sb[hs, t0:t0 + 128],
                                     q_sb[hs, tq0 + c0:tq0 + tb],
                                     start=True, stop=True)
                return ps_s, p_t

            def attn_block(b, qb, pre_qk=None, next_unit=None):
                tq0 = b * seq + qb * tb
                ntk = (qb + 1) * dtiles
                # flipped-PV accumulators: token-major y per head,
                # [128 tq, 4 subtiles x (hd+1)]; col hd of each subtile = denom
                ps_yt = [psum.tile([128, dtiles * (hd + 1)], F32, name=f"ps_yt{h}",
                                   tag=f"yt{h}", bufs=1) for h in range(hpc)]
                yt_sb = work.tile([128, dtiles * fl], BF16, name="yt_sb",
                                  tag="ytsb", bufs=4)

                def norm_all():
                    # all subtiles accumulated (bank group stopped): normalize
                    # token-major, one broadcast multiply per head
                    for h in range(hpc):
                        yv = ps_yt[h][:].rearrange("p (s c) -> p s c", c=hd + 1)
                        recip = work.tile([128, dtiles], F32, name="recip",
                                          tag="recip", bufs=2)
                        rv = recip[:].rearrange("p (s o) -> p s o", o=1)
                        nc.vector.reciprocal(rv, yv[:, :, hd:hd + 1])
                        ytv = yt_sb[:].rearrange(
                            "p (s f) -> p s f", f=fl)[:, :, h * hd:(h + 1) * hd]
                        nc.vector.tensor_mul(
                            ytv, yv[:, :, 0:hd],
                            rv.broadcast_to((128, dtiles, hd)))

                def transpose_sub(s):
                    ps_tr = psum.tile([128, 128], BF16, name="ps_tr",
                                      tag="mm512", bufs=2)
                    nc.tensor.transpose(ps_tr[:], yt_sb[:, s * fl:(s + 1) * fl],
                                        ident[:])
                    nc.vector.tensor_copy(
                        y_sb[:, tq0 + s * 128:tq0 + (s + 1) * 128], ps_tr[:])

                def c0_of(tki):
                    m = tki - qb * dtiles
                    return 128 * m if m > 0 else 0

                cur = pre_qk if pre_qk is not None else qk_scores(b, qb, 0)
                next_qk = None
                for tki in range(ntk):
                    gti = (b * seq) // 128 + tki
                    m = tki - qb * dtiles
                    c0 = c0_of(tki)
                    ps_s, p_t = cur
                    # one exp for both heads: AP [128, hpc, nq]
                    sv = ps_s[:].rearrange("p (h q) -> p h q", h=hpc)[:, :, c0:tb]
                    pv = p_t[:].rearrange("p (h q) -> p h q", h=hpc)[:, :, c0:tb]
                    nc.scalar.activation(pv, sv, AF.Exp, scale=scale)
                    # software pipeline: next QK on the PE queue BEFORE this
                    # k-tile's PV (which blocks on exp+mask) so the exp stream
                    # stays saturated; at block end, prefetch the NEXT unit's
                    # first QK
                    if tki + 1 < ntk:
                        cur = qk_scores(b, qb, tki + 1)
                    elif next_unit is not None:
                        next_qk = qk_scores(*next_unit, 0)
                    if m >= 0:
                        # mask the [128,128] triangle at cols [c0, c0+128),
                        # both heads in one op (tri broadcast over heads)
                        ap = p_t[:].rearrange(
                            "p (h q) -> p h q", h=hpc)[:, :, c0:c0 + 128]
                        nc.vector.tensor_mul(
                            ap, ap,
                            tri[:].unsqueeze(1).broadcast_to((128, hpc, 128)))
                    # flipped PV: per q-subtile s, stationary = P [tk, tq=128],
                    # moving = V+ones [tk, hd+1] -> += y_t [tq, hd+1].
                    # One accumulation group per head-bank: matmul start zeroes
                    # the WHOLE 2KB psum bank, so only the first matmul starts
                    # (pre-zeroing all subtiles) and only the last one stops;
                    # psum reads must wait for the stop, so all finalizes are
                    # emitted after the loop.
                    for h in range(hpc):
                        vm = v_sb[:, gti * vw + h * (hd + 1):gti * vw + (h + 1) * (hd + 1)]
                        for s in range(dtiles):
                            if s < m:
                                continue
                            nc.tensor.matmul(
                                ps_yt[h][:, s * (hd + 1):(s + 1) * (hd + 1)],
                                p_t[:, h * tb + s * 128:h * tb + (s + 1) * 128],
                                vm,
                                start=(tki == 0 and s == 0),
                                stop=(tki == ntk - 1))
                    drain(3 if ntk <= dtiles else (2 if ntk <= 2 * dtiles else 1))
                last = (b, qb) == (bsz - 1, nqb - 1)
                if not last:
                    norm_all()
                    halves = [(0, dtiles)]
                else:
                    # final unit: normalize per subtile-pair so the last
                    # collective's staging DMAs start as soon as possible
                    halves = [(0, 2), (2, dtiles)]
                for s0, s1 in halves:
                    if last:
                        for h in range(hpc):
                            yv = ps_yt[h][:].rearrange("p (s c) -> p s c",
                                                       c=hd + 1)
                            recip = work.tile([128, 2], F32, name="reciph",
                                              tag="reciph", bufs=2)
                            rv = recip[:].rearrange("p (s o) -> p s o", o=1)
                            nc.vector.reciprocal(rv, yv[:, s0:s1, hd:hd + 1])
                            ytv = yt_sb[:].rearrange(
                                "p (s f) -> p s f",
                                f=fl)[:, s0:s1, h * hd:(h + 1) * hd]
                            nc.vector.tensor_mul(
                                ytv, yv[:, s0:s1, 0:hd],
                                rv.broadcast_to((128, s1 - s0, hd)))
                    for s in range(s0, s1):
                        j = 2 * qb + s // 2
                        r0 = (s % 2) * 128
                        eng = nc.scalar if (last and s >= 2) else nc.sync
                        eng.dma_start(
                            a2a_in[b][j][r0:r0 + 128, :],
                            yt_sb[:, s * 128:(s + 1) * 128])
                return next_qk

            def a2a_issue(b):
                nc.gpsimd.collective_compute(
                    "AllToAll", mybir.AluOpType.bypass,
                    replica_groups=[list(range(n_cores))],
                    ins=[a2a_in[b][:]], outs=[a2a_out[b][:]],
                )

            def push_proj_chunks(pb):
                y_loc = [work.tile([fl, tsb], BF16, name="y_loc",
                                   tag=f"yloc{i}", bufs=4) for i in range(n_cores)]
                # transposing DMA loads y_loc feature-major directly from
                # the token-major exchange (XBAR: 16x128 tiles, 14ns each) --
                # no PE transposes, psum traffic, or copies in the tail
                for i in range(n_cores):
                    eng = nc.scalar if (pb == bsz - 1 and i % 2) else nc.sync
                    eng.dma_start_transpose(y_loc[i][:], a2a_out[pb][i])
                for dj in range(d // 128):
                    def p_c(dj=dj):
                        ps_o = psum.tile([128, tsb], F32, name="ps_o",
                                         tag="mm512", bufs=2)
                        for i in range(n_cores):
                            nc.tensor.matmul(
                                ps_o[:], wp_sb[i][:, dj * 128:(dj + 1) * 128],
                                y_loc[i][:], start=(i == 0),
                                stop=(i == n_cores - 1))
                        o_sb = work.tile([128, tsb], F32, name="o_sb",
                                         tag="osb", bufs=6)
                        nc.vector.tensor_copy(o_sb[:], ps_o[:])
                        nc.sync.dma_start(
                            out[dj * 128:(dj + 1) * 128,
                                pb * tsb:(pb + 1) * tsb],
                            o_sb[:])

                    pending_proj.append(p_c)

            units = [(b, qb) for b in range(bsz) for qb in range(nqb)]
            # prologue: unit 0 projected up front; unit 1's x in flight
            x0 = issue_x_dma(*units[0], split_x=True)
            xts = {1: issue_x_dma(*units[1])}
            push_qkv_chunks(*units[0], x0)
            drain()
            pre_qk = None
            for L, (b, qb) in enumerate(units):
                if L >= 1:
                    drain()  # finish this unit's qkv before attending to it
                if L + 1 < len(units):
                    push_qkv_chunks(*units[L + 1], xts.pop(L + 1))
                if L + 2 < len(units):
                    xts[L + 2] = issue_x_dma(*units[L + 2])
                if L == 3:
                    # w_proj loads on the SP queue, after the early x tiles
                    # (first needed when the first proj chunks run, ~90us in)
                    for i in range(n_cores):
                        nc.sync.dma_start(wp_sb[i][:],
                                          wproj[i * fl:(i + 1) * fl, :])
                nxt = units[L + 1] if L + 1 < len(units) else None
                pre_qk = attn_block(b, qb, pre_qk=pre_qk, next_unit=nxt)
                if qb == nqb - 1:
                    a2a_issue(b)
                    if b >= 1:
                        push_proj_chunks(b - 1)
            drain()
            # drain earlier batches' proj first: pushing batch 3 beforehand
            # would enqueue its collective-gated y_loc DMAs ahead of their
            # out-stores on the in-order SP queue, stalling the whole tail
            while pending_proj:
                pending_proj.pop(0)()
            push_proj_chunks(bsz - 1)
            while pending_proj:
                pending_proj.pop(0)()
    _split_multi_waits(nc)
    return nc


def shard_inputs(x, w_qkv, w_proj, n_cores=N_CORES, n_head=N_HEAD):
    bf16 = ml_dtypes.bfloat16
    d = x.shape[-1]
    T = x.shape[0] * x.shape[1]
    hpc = n_head // n_cores
    fl = hpc * HD
    xT = np.ascontiguousarray(np.asarray(x, np.float32).reshape(T, d).T.astype(bf16))
    wq = np.asarray(w_qkv, np.float32)
    wp = np.ascontiguousarray(np.asarray(w_proj, np.float32).T.astype(bf16))
    in_maps = []
    for c in range(n_cores):
        r0 = c * fl
        wqkv_c = np.ascontiguousarray(
            np.concatenate([wq[r0:r0 + fl], wq[d + r0:d + r0 + fl],
                            wq[2 * d + r0:2 * d + r0 + fl]], axis=0).T.astype(bf16))
        in_maps.append({"xT": xT, "wqkv": wqkv_c, "wproj": wp})
    return in_maps


def assemble_out(outs, n_cores=N_CORES, bsz=BSZ, seq=SEQ, d=D):
    """outs[c] is [d, bsz*tsb]; column block b holds tokens
    b*seq + [c*tsb, (c+1)*tsb)."""
    tsb = seq // n_cores
    T = bsz * seq
    outT = np.empty((d, T), np.float32)
    for c in range(n_cores):
        for b in range(bsz):
            outT[:, b * seq + c * tsb:b * seq + (c + 1) * tsb] = \
                outs[c][:, b * tsb:(b + 1) * tsb]
    return np.ascontiguousarray(outT.T).reshape(bsz, seq, d)


_NC_CACHE = {}


def kernel(x, w_qkv, w_proj):
    key = "full"
    if key not in _NC_CACHE:
        _NC_CACHE[key] = build_nc()
    nc = _NC_CACHE[key]
    in_maps = shard_inputs(x, w_qkv, w_proj)
    res = run_bass_kernel_spmd(nc, in_maps, list(range(N_CORES))).results
    return assemble_out([res[c]["out"] for c in range(N_CORES)]).astype(np.float32)
